# revision 13
# baseline (speedup 1.0000x reference)
"""Trainium2 Bass kernel for nn_BasicBlock (MoE-combined residual conv block).

  out = relu(bn2(conv3x3(relu(bn1(conv3x3(x, w1e))), w2e)) + x)
  w{1,2}e = sum_e alpha[e] * w{1,2}[e]   (host-side: linear in weights)

Strategy (per NeuronCore, data-parallel over batch: 32 imgs -> 4 per core x 8):
  - Each conv input lives in SBUF as zero-padded fp16 planes holding four
    row/col-shifted copies of the same image across two [128, 114*114] tiles:
      tile1: partitions 0-63  = pad(x)             ("A")
             partitions 64-127 = A shifted up 1 row (A[r+1])
      tile2: partitions 0-63  = A shifted up 2 rows (A[r+2])
             partitions 64-127 = A up 2 rows, left 1 col (A[r+2,c+1])
    A 3x3 conv then needs only 5 matmuls per output tile instead of 9:
      3 K=128 pairs on tile1 (tap rows 0+1, dw=0,1,2)
      1 K=128 pair  on tile2 (taps (2,0)+(2,1))
      1 K=64  single on tile2 upper (tap (2,2))
  - fp16 matmuls run at 1 cycle/row on the PE; accumulation is fp32 in PSUM.
  - Taps iterate outermost over a group of G=4 PSUM banks so consecutive
    matmuls share stationary weights (LDWEIGHTS amortization).
  - x is loaded once as fp32 in row bands, cast to fp16 on DVE (banded so
    compute starts before the whole image arrives); shifted copies are made
    with SBUF->SBUF DMAs.  conv1 lhsT duplicates the 64 output channels into
    M=128 so PSUM holds two copies; two ACT ops (bn1+relu fused, bn1 scale
    folded into w1) evict them into the A / A>>1row halves of the mid tile1;
    mid tile2 is built by two SBUF->SBUF DMAs.
  - conv2 epilogue on DVE: residual add (fp16 x from plane A) straight out
    of PSUM, then relu (+bn2 bias; bn2 scale folded into w2), DMA to HBM.
"""

import numpy as np

import concourse.bass as bass
import concourse.mybir as mybir
import concourse.tile as tile
from concourse import bacc
from concourse.bass_utils import run_bass_kernel_spmd

F32 = mybir.dt.float32
F16 = mybir.dt.float16
AF = mybir.ActivationFunctionType
ALU = mybir.AluOpType

EPS = 1e-5
N_CORES = 8
C = 64   # channels (in == out)
R = 4    # output rows per PSUM chunk
G = 4    # chunks per weight-stationary group (psum banks per conv)
BAND = 16  # x load/cast band rows


def build_nc(B, H, W):
    """Bass program: B images of [64, H, W] per core."""
    Hp, Wp = H + 2, W + 2
    N = R * W                     # psum free size per chunk
    nchunks = H // R
    assert H % R == 0
    band = BAND if H % BAND == 0 else H
    nbands = H // band

    nc = bacc.Bacc("TRN2", target_bir_lowering=False, debug=False,
                   enable_asserts=False, num_devices=N_CORES)

    xin = nc.dram_tensor("xin", [B, C, H, W], F32, kind="ExternalInput").ap()
    w1p_d = nc.dram_tensor("w1p", [128, 3 * 128], F16, kind="ExternalInput").ap()
    w1q_d = nc.dram_tensor("w1q", [128, 128], F16, kind="ExternalInput").ap()
    w1s_d = nc.dram_tensor("w1s", [64, 128], F16, kind="ExternalInput").ap()
    w2p_d = nc.dram_tensor("w2p", [128, 3 * 64], F16, kind="ExternalInput").ap()
    w2q_d = nc.dram_tensor("w2q", [128, 64], F16, kind="ExternalInput").ap()
    w2s_d = nc.dram_tensor("w2s", [64, 64], F16, kind="ExternalInput").ap()
    b1_d = nc.dram_tensor("b1", [128, 1], F32, kind="ExternalInput").ap()
    b2_d = nc.dram_tensor("b2", [64, 1], F32, kind="ExternalInput").ap()
    yout = nc.dram_tensor("yout", [B, C, H, W], F32, kind="ExternalOutput").ap()

    with tile.TileContext(nc) as tc:
        with (
            tc.tile_pool(name="wpool", bufs=1) as wpool,
            tc.tile_pool(name="xpool", bufs=2) as xpool,
            tc.tile_pool(name="x2pool", bufs=2) as x2pool,
            tc.tile_pool(name="fpool", bufs=2) as fpool,
            tc.tile_pool(name="mpool", bufs=1) as mpool,
            tc.tile_pool(name="m2pool", bufs=1) as m2pool,
            tc.tile_pool(name="p1pool", bufs=G, space="PSUM") as p1pool,
            tc.tile_pool(name="p2pool", bufs=G, space="PSUM") as p2pool,
            tc.tile_pool(name="upool", bufs=4) as upool,
            tc.tile_pool(name="opool", bufs=4) as opool,
        ):
            w1p = wpool.tile([128, 3 * 128], F16)
            w1q = wpool.tile([128, 128], F16)
            w1s = wpool.tile([128, 128], F16)  # single lives in parts 64-127
            w2p = wpool.tile([128, 3 * 64], F16)
            w2q = wpool.tile([128, 64], F16)
            w2s = wpool.tile([128, 64], F16)
            b1t = wpool.tile([128, 1], F32)
            b2t = wpool.tile([64, 1], F32)
            nc.sync.dma_start(w1p[:, :], w1p_d[:, :])
            nc.sync.dma_start(w1q[:, :], w1q_d[:, :])
            nc.sync.dma_start(w1s[64:128, :], w1s_d[:, :])
            nc.sync.dma_start(w2p[:, :], w2p_d[:, :])
            nc.sync.dma_start(w2q[:, :], w2q_d[:, :])
            nc.sync.dma_start(w2s[64:128, :], w2s_d[:, :])
            nc.sync.dma_start(b1t[:, :], b1_d[:, :])
            nc.sync.dma_start(b2t[:, :], b2_d[:, :])

            def x_prep(img):
                """Emit input-plane construction for one image; returns views."""
                xt = xpool.tile([128, Hp * Wp], F16, tag="xt",
                                name=f"xt_{img}")
                xr = xt[:, :].rearrange("p (h w) -> p h w", w=Wp)
                xt2 = x2pool.tile([128, Hp * Wp], F16, tag="xt2",
                                  name=f"xt2_{img}")
                x2r = xt2[:, :].rearrange("p (h w) -> p h w", w=Wp)
                nc.vector.memset(xr[0:64, 0, :], 0.0)          # A top border
                nc.vector.memset(xr[0:64, Hp - 1, :], 0.0)     # A bottom border
                nc.vector.memset(xr[0:64, :, 0], 0.0)          # A left border
                nc.vector.memset(xr[0:64, :, Wp - 1], 0.0)     # A right border
                nc.vector.memset(xr[64:128, H, :], 0.0)        # A>>1 bottom
                # banded: f32 load -> DVE cast into A -> shifted SBUF copies
                xin_r = xin[img]
                for b in range(nbands):
                    r0 = b * band
                    xf = fpool.tile([64, band * W], F32, tag="xf",
                                    name=f"xf_{img}_{b}")
                    nc.sync.dma_start(
                        xf[:, :].rearrange("p (h w) -> p h w", w=W),
                        xin_r[:, r0:r0 + band, :])
                    nc.vector.tensor_copy(
                        xr[0:64, r0 + 1:r0 + band + 1, 1:W + 1],
                        xf[:, :].rearrange("p (h w) -> p h w", w=W))
                    # A>>1row band: rows r0..r0+band-1 <- A rows r0+1..
                    nc.sync.dma_start(xr[64:128, r0:r0 + band, :],
                                      xr[0:64, r0 + 1:r0 + band + 1, :])
                    if b > 0:
                        p0 = (b - 1) * band
                        nc.sync.dma_start(x2r[0:64, p0:p0 + band, :],
                                          xr[0:64, p0 + 2:p0 + band + 2, :])
                        nc.sync.dma_start(x2r[64:128, p0:p0 + band, 0:Wp - 1],
                                          xr[0:64, p0 + 2:p0 + band + 2, 1:Wp])
                p0 = (nbands - 1) * band
                nc.sync.dma_start(x2r[0:64, p0:p0 + band, :],
                                  xr[0:64, p0 + 2:p0 + band + 2, :])
                nc.sync.dma_start(x2r[64:128, p0:p0 + band, 0:Wp - 1],
                                  xr[0:64, p0 + 2:p0 + band + 2, 1:Wp])
                return xr, x2r

            views = x_prep(0)
            for img in range(B):
                xr, x2r = views

                # ---- mid planes ----
                mt = mpool.tile([128, Hp * Wp], F16, tag="mt")
                mr = mt[:, :].rearrange("p (h w) -> p h w", w=Wp)
                mt2 = m2pool.tile([128, Hp * Wp], F16, tag="mt2")
                m2r = mt2[:, :].rearrange("p (h w) -> p h w", w=Wp)
                nc.vector.memset(mr[0:64, 0, :], 0.0)
                nc.vector.memset(mr[0:64, Hp - 1, :], 0.0)
                nc.vector.memset(mr[64:128, H, :], 0.0)
                nc.vector.memset(mr[:, :, 0], 0.0)
                nc.vector.memset(mr[:, :, Wp - 1], 0.0)

                # ---- conv1 + bn1 + relu -> mid (taps outer over G banks) ----
                for g0 in range(0, nchunks, G):
                    ng = min(G, nchunks - g0)
                    pss = [p1pool.tile([128, N], F32, tag="ps1",
                                       name=f"ps1_{img}_{g0}_{j}")
                           for j in range(ng)]
                    for dw in range(3):
                        for j in range(ng):
                            h0 = (g0 + j) * R
                            nc.tensor.matmul(
                                pss[j][:, :],
                                lhsT=w1p[:, dw * 128:(dw + 1) * 128],
                                rhs=xr[0:128, h0:h0 + R, dw:dw + W],
                                start=(dw == 0), stop=False)
                    for j in range(ng):
                        h0 = (g0 + j) * R
                        nc.tensor.matmul(
                            pss[j][:, :], lhsT=w1q[:, :],
                            rhs=x2r[0:128, h0:h0 + R, 0:W],
                            start=False, stop=False)
                    for j in range(ng):
                        h0 = (g0 + j) * R
                        nc.tensor.matmul(
                            pss[j][:, :], lhsT=w1s[64:128, :],
                            rhs=x2r[64:128, h0:h0 + R, 1:1 + W],
                            start=False, stop=True)
                    for j in range(ng):
                        h0 = (g0 + j) * R
                        ps1 = pss[j]
                        p1lo = ps1[0:64, :].rearrange("p (h w) -> p h w", w=W)
                        p1hi = ps1[64:128, :].rearrange("p (h w) -> p h w", w=W)
                        nc.scalar.activation(
                            mr[0:64, h0 + 1:h0 + 1 + R, 1:W + 1],
                            p1lo, AF.Relu, bias=b1t[0:64, 0:1])
                        nc.scalar.activation(
                            mr[64:128, h0:h0 + R, 1:W + 1],
                            p1hi, AF.Relu, bias=b1t[64:128, 0:1])

                # prefetch next image's input planes while conv2 runs
                if img + 1 < B:
                    views = x_prep(img + 1)

                # mid tile2 = A>>2rows (+1 col in upper half)
                nc.sync.dma_start(m2r[0:64, 0:H, :], mr[0:64, 2:Hp, :])
                nc.sync.dma_start(m2r[64:128, 0:H, 0:Wp - 1],
                                  mr[0:64, 2:Hp, 1:Wp])

                # ---- conv2 + bn2 + residual + relu -> out ----
                for g0 in range(0, nchunks, G):
                    ng = min(G, nchunks - g0)
                    pss = [p2pool.tile([64, N], F32, tag="ps2",
                                       name=f"ps2_{img}_{g0}_{j}")
                           for j in range(ng)]
                    for dw in range(3):
                        for j in range(ng):
                            h0 = (g0 + j) * R
                            nc.tensor.matmul(
                                pss[j][:, :],
                                lhsT=w2p[:, dw * 64:(dw + 1) * 64],
                                rhs=mr[0:128, h0:h0 + R, dw:dw + W],
                                start=(dw == 0), stop=False)
                    for j in range(ng):
                        h0 = (g0 + j) * R
                        nc.tensor.matmul(
                            pss[j][:, :], lhsT=w2q[:, :],
                            rhs=m2r[0:128, h0:h0 + R, 0:W],
                            start=False, stop=False)
                    for j in range(ng):
                        h0 = (g0 + j) * R
                        nc.tensor.matmul(
                            pss[j][:, :], lhsT=w2s[64:128, :],
                            rhs=m2r[64:128, h0:h0 + R, 1:1 + W],
                            start=False, stop=True)
                    for j in range(ng):
                        h0 = (g0 + j) * R
                        ps2 = pss[j]
                        u = upool.tile([64, N], F32, tag="u",
                                       name=f"u_{img}_{g0}_{j}")
                        nc.vector.tensor_add(
                            u[:, :].rearrange("p (h w) -> p h w", w=W),
                            ps2[:, :].rearrange("p (h w) -> p h w", w=W),
                            xr[0:64, h0 + 1:h0 + 1 + R, 1:W + 1])
                        o = opool.tile([64, N], F32, tag="o",
                                       name=f"o_{img}_{g0}_{j}")
                        nc.vector.tensor_scalar(
                            o[:, :], u[:, :], b2t[:, 0:1], 0.0,
                            ALU.add, ALU.max)
                        nc.sync.dma_start(
                            yout[img][:, h0:h0 + R, :],
                            o[:, :].rearrange("p (h w) -> p h w", w=W))
    nc.compile()
    return nc


def prepare_weights(w1, w2, alpha, bn1_gamma, bn1_beta, bn1_mean, bn1_var,
                    bn2_gamma, bn2_beta, bn2_mean, bn2_var):
    w1e = np.einsum('e,eoihw->oihw', alpha.astype(np.float64),
                    w1.astype(np.float64))
    w2e = np.einsum('e,eoihw->oihw', alpha.astype(np.float64),
                    w2.astype(np.float64))
    s1 = bn1_gamma / np.sqrt(bn1_var + EPS)
    b1 = bn1_beta - bn1_mean * s1
    s2 = bn2_gamma / np.sqrt(bn2_var + EPS)
    b2 = bn2_beta - bn2_mean * s2
    w1e = (w1e * s1[:, None, None, None]).astype(np.float16)  # fold bn1 scale
    w2e = (w2e * s2[:, None, None, None]).astype(np.float16)  # fold bn2 scale

    w1p = np.zeros((128, 3 * 128), np.float16)
    w1q = np.zeros((128, 128), np.float16)
    w1s = np.zeros((64, 128), np.float16)
    w2p = np.zeros((128, 3 * 64), np.float16)
    w2q = np.zeros((128, 64), np.float16)
    w2s = np.zeros((64, 64), np.float16)
    for dw in range(3):
        for dh in (0, 1):
            w1p[dh * 64:(dh + 1) * 64, dw * 128:dw * 128 + 64] = w1e[:, :, dh, dw].T
            w1p[dh * 64:(dh + 1) * 64, dw * 128 + 64:dw * 128 + 128] = w1e[:, :, dh, dw].T
            w2p[dh * 64:(dh + 1) * 64, dw * 64:(dw + 1) * 64] = w2e[:, :, dh, dw].T
    # tile2 pair: tap (2,0) weights in lower partitions, (2,1) in upper
    w1q[0:64, 0:64] = w1e[:, :, 2, 0].T
    w1q[0:64, 64:128] = w1e[:, :, 2, 0].T
    w1q[64:128, 0:64] = w1e[:, :, 2, 1].T
    w1q[64:128, 64:128] = w1e[:, :, 2, 1].T
    w1s[:, 0:64] = w1e[:, :, 2, 2].T
    w1s[:, 64:128] = w1e[:, :, 2, 2].T
    w2q[0:64, :] = w2e[:, :, 2, 0].T
    w2q[64:128, :] = w2e[:, :, 2, 1].T
    w2s[:, :] = w2e[:, :, 2, 2].T
    b1v = np.tile(b1.astype(np.float32), 2).reshape(128, 1)
    b2v = b2.astype(np.float32).reshape(64, 1)
    return w1p, w1q, w1s, w2p, w2q, w2s, b1v, b2v


_NC_CACHE = {}


def kernel(x, w1, w2, alpha,
           bn1_gamma, bn1_beta, bn1_mean, bn1_var,
           bn2_gamma, bn2_beta, bn2_mean, bn2_var):
    x = np.ascontiguousarray(np.asarray(x, dtype=np.float32))
    B_total, _, H, W = x.shape
    Bc = B_total // N_CORES
    w1p, w1q, w1s, w2p, w2q, w2s, b1v, b2v = prepare_weights(
        np.asarray(w1, np.float32), np.asarray(w2, np.float32),
        np.asarray(alpha, np.float32),
        np.asarray(bn1_gamma, np.float32), np.asarray(bn1_beta, np.float32),
        np.asarray(bn1_mean, np.float32), np.asarray(bn1_var, np.float32),
        np.asarray(bn2_gamma, np.float32), np.asarray(bn2_beta, np.float32),
        np.asarray(bn2_mean, np.float32), np.asarray(bn2_var, np.float32))

    key = (Bc, H, W)
    if key not in _NC_CACHE:
        _NC_CACHE[key] = build_nc(Bc, H, W)
    nc = _NC_CACHE[key]

    in_maps = []
    for cid in range(N_CORES):
        in_maps.append({
            "xin": x[cid * Bc:(cid + 1) * Bc],
            "w1p": w1p, "w1q": w1q, "w1s": w1s,
            "w2p": w2p, "w2q": w2q, "w2s": w2s,
            "b1": b1v, "b2": b2v,
        })
    res = run_bass_kernel_spmd(nc, in_maps, core_ids=list(range(N_CORES)))
    out = np.concatenate([res.results[cid]["yout"] for cid in range(N_CORES)],
                         axis=0)
    return out
